# revision 50
# baseline (speedup 1.0000x reference)
"""Trainium2 Bass kernel for nn_DGLGraphConv (graph conv with sum- and product-reduce).

Strategy (8 NeuronCores, SPMD, two launches):
  Launch A (node-sharded, 6250 nodes/core): per-node table, bf16, 256 cols:
      T[n] = [ (feat@w1)*s_out (128) | |log|tanh|| (64) | neg indicator (64) ]
  Host: concat T shards (pure relayout; row 0 / tail rows are zeros used as gather padding).
  Launch B (dst-sharded by edge partitioning): per core, 49 windows x 128 dst slots.
      Edges sorted by (dst window, src half, src), padded to 128-edge blocks with
      block counts maxed over cores so all cores run one program. Per block:
      dma_gather 128 rows of T (512B each), one-hot S matmul accumulates
      [sum(fs) | sum|ln| | count(neg)] per dst slot in PSUM; epilogue computes
      sign via parity, exp, mask, @v, and the in-degree scaling.

Host does integer index prep (bincount/sort/pad) and layout-only transforms;
all floating-point math runs on device.
"""
import sys
from contextlib import ExitStack

import numpy as np

for _p in ("/opt/trn_rl_repo",):
    if _p not in sys.path:
        sys.path.insert(0, _p)

import concourse.bass as bass
import concourse.mybir as mybir
import concourse.tile as tile
from concourse import bacc, bass_utils
from concourse._compat import with_exitstack
from concourse.masks import make_identity

F32 = mybir.dt.float32
I32 = mybir.dt.int32
I16 = mybir.dt.int16
BF16 = mybir.dt.bfloat16
AF = mybir.ActivationFunctionType
ALU = mybir.AluOpType

MASK_ABS = 0x7FFFFFFF
MASK_SGN = -0x80000000

# T table row stride (bf16 elems, 512B — gather table stride must be a
# multiple of 256B); payload is TCOL cols:
#   [fs(128) | absl(64) | negpk(32)]  negpk[k] = neg[k] + 128*neg[k+32]
TROW = 256
TCOL = 224
import os as _os
# Max 128-edge blocks per dma_gather call. Full-window calls (~2200 emitted
# descriptors) die on HW (descriptor-ring overflow); pad slots are emitted as
# trailing -1 (skipped by the ucode), keeping per-call emission ~<=1450.
GATHER_CHUNK = int(_os.environ.get("GCH", "6"))
SINGLE_PACKET = _os.environ.get("SPKT", "0") == "1"
# The gather ucode runs on Q7 core pair `queue_num` (cpu_id/2 == queue_num):
# 4 SWDGE queues = 4 concurrent descriptor-generation pairs. Balance calls
# over queues greedily by descriptor count.
NQUEUE = int(_os.environ.get("NQ", "4"))  # rotate gather calls over SWDGE queues
NSWQ = int(_os.environ.get("NSWQ", "4"))  # SWDGE queues allocated (1..4)
GROUP_W = int(_os.environ.get("GROUPW", "12"))  # windows per epilogue group
GBUFD = int(_os.environ.get("GBUFD", "4"))  # gather-buffer rotation depth
# windows whose G buffer may be uninitialized SBUF (NaN patterns): gather real
# zero rows for padding there instead of skipping, so 0*NaN never reaches PSUM.
PAD_REAL_WINDOWS = 3


# ---------------- host-side prep (integer/layout only) ----------------

def make_dims(N=50000, E=800000, DIN=256, DOUT=128, RANK=64, M=8, LO_MAX=32766):
    LO_MAX = min(LO_MAX, N - 1)
    NSH = N // M
    W = 128
    NW = (NSH + W - 1) // W
    T_ROWS = N + 4
    HI_BASE = LO_MAX + 2
    NHI = T_ROWS - HI_BASE
    return dict(N=N, E=E, DIN=DIN, DOUT=DOUT, RANK=RANK, M=M, NSH=NSH, W=W, NW=NW,
                LO_MAX=LO_MAX, T_ROWS=T_ROWS, HI_BASE=HI_BASE, NHI=NHI,
                HI_PAD_IDX=NHI - 2)


def preprocess(src, dst, dm):
    N, E, M, NSH, W, NW = dm["N"], dm["E"], dm["M"], dm["NSH"], dm["W"], dm["NW"]
    LO_MAX, HI_PAD_IDX = dm["LO_MAX"], dm["HI_PAD_IDX"]
    src = np.asarray(src).astype(np.int64)
    dst = np.asarray(dst).astype(np.int64)
    deg_out = np.bincount(src, minlength=N).astype(np.float32)
    deg_in = np.bincount(dst, minlength=N).astype(np.float32)

    order = np.lexsort((src, dst))
    s_srt, d_srt = src[order], dst[order]
    core_of = d_srt // NSH
    win_of = (d_srt - core_of * NSH) // W

    is_hi = s_srt > LO_MAX
    nlo = np.zeros((M, NW), np.int64)
    nhi = np.zeros((M, NW), np.int64)
    np.add.at(nlo, (core_of[~is_hi], win_of[~is_hi]), 1)
    np.add.at(nhi, (core_of[is_hi], win_of[is_hi]), 1)

    BL = np.maximum(1, -(-nlo.max(axis=0) // 128))
    BH = -(-nhi.max(axis=0) // 128)
    BT = BL + BH
    NB = int(BT.sum())

    idx16 = np.zeros((M, NB * 128), np.int16)
    dstloc = np.zeros((M, 128, NB), np.float32)
    key_half = is_hi.astype(np.int64)
    order2 = np.lexsort((s_srt, key_half, win_of, core_of))
    s2, d2 = s_srt[order2], d_srt[order2]
    c2, w2_, h2 = core_of[order2], win_of[order2], key_half[order2]

    woff = np.concatenate([[0], np.cumsum(BT)])
    counts = np.zeros((M, NW, 2), np.int64)
    np.add.at(counts, (c2, w2_, h2), 1)
    dstloc[:] = -1.0  # pad slots map to no dst slot (S one-hot row is zero)
    ptr = 0
    gch = GATHER_CHUNK if GATHER_CHUNK > 0 else 10**9
    for k in range(M):
        for w in range(NW):
            cl, ch = counts[k, w, 0], counts[k, w, 1]
            base_blk = woff[w]
            seg = slice(ptr, ptr + cl)
            pos = np.arange(cl)
            idx16[k, base_blk * 128 + pos] = (s2[seg] + 1).astype(np.int16)
            dstloc[k, pos % 128, base_blk + pos // 128] = (d2[seg] - k * NSH - w * W).astype(np.float32)
            ptr += cl
            pad_real = True  # -1 skip-pads crash the gather ucode on HW
            if cl < BL[w] * 128:
                pad = np.arange(cl, BL[w] * 128)
                idx16[k, base_blk * 128 + pad] = 0 if pad_real else -1
            base_blk_h = woff[w] + BL[w]
            seg = slice(ptr, ptr + ch)
            pos = np.arange(ch)
            idx16[k, base_blk_h * 128 + pos] = (s2[seg] - LO_MAX - 1).astype(np.int16)
            dstloc[k, pos % 128, base_blk_h + pos // 128] = (d2[seg] - k * NSH - w * W).astype(np.float32)
            ptr += ch
            if ch < BH[w] * 128:
                pad = np.arange(ch, BH[w] * 128)
                idx16[k, base_blk_h * 128 + pad] = HI_PAD_IDX if pad_real else -1
            # each of the 16 wrapped index lanes maps to one SDMA engine and
            # the DMA completion semaphore needs every engine to emit: keep
            # >=16 real (non-negative) indices at the head of every call.
            for sec_lo, sec_nb, fill in ((base_blk, int(BL[w]), 0),
                                         (base_blk_h, int(BH[w]), HI_PAD_IDX)):
                for s in range(0, sec_nb, gch):
                    e = min(s + gch, sec_nb)
                    sl = idx16[k, (sec_lo + s) * 128:(sec_lo + e) * 128]
                    if (sl[:16] < 0).any():
                        sl[:16][sl[:16] < 0] = fill
    assert ptr == E

    idx16_wrapped = np.tile(idx16.reshape(M, -1, 16).transpose(0, 2, 1), (1, 8, 1)).copy()

    def shard_deg(deg):
        out = np.zeros((M, 128, NW), np.float32)
        for k in range(M):
            d = deg[k * NSH:(k + 1) * NSH]
            d = np.concatenate([d, np.zeros(NW * W - NSH, np.float32)])
            out[k] = d.reshape(NW, W).T
        return out

    return dict(BL=BL, BH=BH, BT=BT, NB=NB, idx16_wrapped=idx16_wrapped, dstloc=dstloc,
                deg_in_sh=shard_deg(deg_in), deg_out=deg_out, woff=woff,
                ni_lo=counts[:, :, 0].max(axis=0), ni_hi=counts[:, :, 1].max(axis=0),
                cnt_own=counts.astype(np.int32))


def host_phase_a_inputs(feat, w1, w2, pp, dm, deg_out):
    M, NSH, DIN = dm["M"], dm["NSH"], dm["DIN"]
    featT = np.ascontiguousarray(np.asarray(feat).T)
    Wcat = np.ascontiguousarray(np.concatenate([w1, w2[:DIN]], axis=1), dtype=np.float32)
    b2col = np.ascontiguousarray(w2[DIN][:, None], dtype=np.float32)
    return [dict(featT=np.ascontiguousarray(featT[:, k * NSH:(k + 1) * NSH]),
                 Wcat=Wcat, b2col=b2col,
                 deg_row=np.ascontiguousarray(deg_out[None, k * NSH:(k + 1) * NSH]))
            for k in range(M)]


def assemble_T(shards, dm):
    # shards are [TCOL, NSH] (feature-major); T rows are nodes, padded to a
    # TROW (512B) stride — pure relayout.
    width = shards[0].shape[0]
    T = np.zeros((dm["T_ROWS"], TROW), shards[0].dtype)
    for k in range(dm["M"]):
        T[1 + k * dm["NSH"]:1 + (k + 1) * dm["NSH"], :width] = shards[k].T
    return T


def host_phase_b_inputs(T, pp, vmat, dm):
    NW = dm["NW"]
    # per-core real index counts per (window, section), clipped to the static
    # chunk geometry: >=16 keeps every SDMA lane's completion-sem leg alive.
    stat = np.stack([np.maximum(pp["ni_lo"], 16), np.maximum(pp["ni_hi"], 16)], axis=1)
    outs = []
    for k in range(dm["M"]):
        cnt = np.clip(pp["cnt_own"][k], 16, stat).astype(np.int32).reshape(1, NW * 2)
        outs.append(dict(T=T, idx16w=pp["idx16_wrapped"][k], dstloc=pp["dstloc"][k],
                         deg_in_sh=pp["deg_in_sh"][k],
                         vmat=np.ascontiguousarray(vmat, np.float32), cnt=cnt))
    return outs


# ---------------- device kernels ----------------

def dma_gather_strided(eng, out_ap, in_ap, idxs_ap, num_idxs, num_idxs_reg,
                       elem_size, elem_step, queue_num=0, single_packet=False):
    """nc.gpsimd.dma_gather minus its `elem_size_bytes % 256 == 0` assert.

    Only the table STRIDE (elem_step) must be a 256B multiple (the ucode
    encodes it as stride_bytes_256); the per-row payload moved can be any
    size <= stride. Mirrors the non-transpose DRAM-source tail of
    bass.dma_gather.
    """
    from concourse import ap_utils
    from concourse._compat import exact_div, round_up_to_multiple
    eng._assert_queue_num(queue_num)
    assert idxs_ap.dtype == mybir.dt.int16
    assert in_ap.dtype == out_ap.dtype
    assert ap_utils.ap_is_contiguous(out_ap.ap[1:])
    assert ap_utils.ap_is_contiguous(idxs_ap.ap[1:])
    assert in_ap.ap[-1][1] == out_ap.ap[-1][1] == elem_size
    assert out_ap.ap[0][1] * out_ap.ap[1][1] == round_up_to_multiple(num_idxs, 128)
    assert in_ap.ap[0][0] == elem_step
    stride_bytes = elem_step * mybir.dt.size(in_ap.dtype)
    stride_bytes_256 = exact_div(stride_bytes, 256)
    assert stride_bytes_256 < 256
    _in_ap = eng.lower_ap_dma(in_ap, for_custom_bir_dma=True)
    _idxs_ap = eng.lower_ap(idxs_ap)
    _out_ap = eng.lower_ap(out_ap)
    return eng.add_instruction(
        mybir.InstDMAGatherAnt(
            name=eng.bass.get_next_instruction_name(),
            ins=[*_in_ap, _idxs_ap, eng.lower_val_access(eng.to_reg(num_idxs_reg))],
            outs=[_out_ap],
            transpose=False,
            num_idxs=num_idxs,
            elem_size=elem_size,
            stride_bytes_256=stride_bytes_256,
            gen_mode=0,
            single_packet=single_packet,
            queue_num=queue_num,
            sbuf_tokens_per_rank=0,
            sbuf_free_dim_per_rank=0,
            sbuf_free_dim_pad_per_rank=0,
            sbuf_byte_offset=0,
        ))

@with_exitstack
def build_phase_a(ctx, tc, outs, ins, cfg):
    """Transposed orientation: features on partitions, nodes on the free dim.

    Output Tt [256, NSH] bf16: rows 0:128 fs = (feat*s)@w1, rows 128:192
    ln = log|tanh((feat*s)@w2 + b)| (signed, clamped >= -60), rows 192:256
    the tanh-negative indicator. Host transposes into T rows.
    """
    nc = tc.nc
    NSH = cfg["NSH"]
    DIN = cfg["DIN"]
    DO, RK = cfg["DOUT"], cfg["RANK"]
    DT = DO + RK
    KC = DIN // 128
    CW = 512
    NCHUNK = (NSH + CW - 1) // CW
    NLOAD = 4

    cpool = ctx.enter_context(tc.tile_pool(name="const", bufs=1))
    pp1 = ctx.enter_context(tc.tile_pool(name="ps1", bufs=3, space="PSUM"))
    pp2 = ctx.enter_context(tc.tile_pool(name="ps2", bufs=3, space="PSUM"))

    Wsb32 = cpool.tile([128, KC, DT], F32)
    nc.sync.dma_start(out=Wsb32[:],
                      in_=ins["Wcat"].rearrange("(c p) n -> p c n", p=128))
    Wsb = cpool.tile([128, KC, DT], BF16)
    nc.vector.tensor_copy(out=Wsb[:], in_=Wsb32[:])
    b2 = cpool.tile([64, 1], F32)
    nc.sync.dma_start(out=b2[:], in_=ins["b2col"][:])

    # out-degree scale: replicate deg to all partitions via a 0-stride DRAM
    # read, then clamp + Ln + Exp(-0.5x). Run in halves so the first half's
    # chain (which gates the first chunk multiplies) overlaps the second
    # half's broadcast DMA.
    srep = cpool.tile([128, NSH], F32)
    shalf = NSH // 4
    for h0, h1 in ((0, shalf), (shalf, 2 * shalf), (2 * shalf, 3 * shalf),
                   (3 * shalf, NSH)):
        hs = slice(h0, h1)
        nc.sync.dma_start(out=srep[:, hs],
                          in_=ins["deg_row"][0:1, hs].to_broadcast([128, h1 - h0]))
        nc.vector.tensor_scalar(out=srep[:, hs], in0=srep[:, hs], scalar1=1.0,
                                scalar2=None, op0=ALU.max)
        nc.scalar.activation(out=srep[:, hs], in_=srep[:, hs], func=AF.Ln)
        nc.scalar.activation(out=srep[:, hs], in_=srep[:, hs], func=AF.Exp,
                             scale=-0.5)

    featT = cpool.tile([128, KC, NSH], BF16)
    step = (NSH + NLOAD - 1) // NLOAD
    featT_r = ins["featT"].rearrange("(c p) n -> p c n", p=128)
    for i in range(0, NSH, step):
        j = min(i + step, NSH)
        # SWDGE: only gpsimd DMA can cast f32->bf16 in flight
        nc.gpsimd.dma_start(out=featT[:, :, i:j], in_=featT_r[:, :, i:j])

    sfs = cpool.tile([128, NSH], BF16)
    sln = cpool.tile([64, NSH], BF16)
    sng = cpool.tile([64, NSH], BF16)
    thall = cpool.tile([64, NSH], F32)

    for q in range(NCHUNK):
        i0 = q * CW
        cw = min(CW, NSH - i0)
        sl = slice(i0, i0 + cw)
        ps1 = pp1.tile([128, CW], F32, tag="ps1")
        for c in range(KC):
            nc.tensor.matmul(out=ps1[:, :cw], lhsT=Wsb[:, c, 0:DO],
                             rhs=featT[:, c, sl], start=(c == 0), stop=(c == KC - 1))
        ps2 = pp2.tile([64, CW], F32, tag="ps2")
        for c in range(KC):
            nc.tensor.matmul(out=ps2[:, :cw], lhsT=Wsb[:, c, DO:DT],
                             rhs=featT[:, c, sl], start=(c == 0), stop=(c == KC - 1))
        nc.vector.tensor_tensor(out=sfs[:, sl], in0=ps1[:, :cw], in1=srep[:, sl],
                                op=ALU.mult)
        nc.vector.tensor_tensor(out=thall[:, sl], in0=ps2[:, :cw], in1=srep[0:64, sl],
                                op=ALU.mult)
        # tanh in-loop (single ACT table resident) so it overlaps the matmuls
        nc.scalar.activation(out=thall[:, sl], in_=thall[:, sl], func=AF.Tanh,
                             bias=b2[:])

    # sfs is final once the loop drains — write it while the log chain runs
    half = NSH // 2
    nc.sync.dma_start(out=outs["Tsh"][0:DO, 0:half], in_=sfs[:, 0:half])
    nc.sync.dma_start(out=outs["Tsh"][0:DO, half:NSH], in_=sfs[:, half:NSH])

    # hoisted log chain (one Ln table load), in halves so the stages and the
    # output DMAs pipeline; npk = a + 128*b ({0,1,128,129} bf16-exact).
    npk = cpool.tile([32, NSH], BF16)
    for h0, h1 in ((0, half), (half, NSH)):
        hs = slice(h0, h1)
        nc.vector.tensor_scalar(out=sng[:, hs], in0=thall[:, hs],
                                scalar1=0.0, scalar2=None, op0=ALU.is_lt)
        nc.vector.tensor_scalar(out=thall[:, hs].bitcast(I32),
                                in0=thall[:, hs].bitcast(I32),
                                scalar1=MASK_ABS, scalar2=None, op0=ALU.bitwise_and)
        nc.scalar.activation(out=thall[:, hs], in_=thall[:, hs], func=AF.Ln)
        nc.vector.tensor_scalar(out=sln[:, hs], in0=thall[:, hs],
                                scalar1=-60.0, scalar2=None, op0=ALU.max)
        nc.vector.tensor_scalar(out=npk[:, hs], in0=sng[32:64, hs], scalar1=128.0,
                                scalar2=None, op0=ALU.mult)
        nc.vector.tensor_tensor(out=npk[:, hs], in0=npk[:, hs], in1=sng[0:32, hs],
                                op=ALU.add)
        nc.sync.dma_start(out=outs["Tsh"][DO:DT, hs], in_=sln[:, hs])
        nc.sync.dma_start(out=outs["Tsh"][DT:TCOL, hs], in_=npk[:, hs])


@with_exitstack
def build_phase_b(ctx, tc, outs, ins, cfg):
    nc = tc.nc
    NW = cfg["NW"]
    DO, RK = cfg["DOUT"], cfg["RANK"]
    DT = DO + RK
    BL, BH = cfg["BL"], cfg["BH"]
    woff = cfg["woff"]
    NB = cfg["NB"]
    HI_BASE = cfg["HI_BASE"]
    T_ROWS = cfg["T_ROWS"]
    GROUP = cfg.get("GROUP", 12)
    GBD = GBUFD
    NBMAX = int(max(BL[w] + BH[w] for w in range(NW)))

    cpool = ctx.enter_context(tc.tile_pool(name="const", bufs=1))
    spool = ctx.enter_context(tc.tile_pool(name="s", bufs=2))
    hpool = ctx.enter_context(tc.tile_pool(name="h", bufs=2))
    epool = ctx.enter_context(tc.tile_pool(name="e", bufs=2))
    rpool = ctx.enter_context(tc.tile_pool(name="r", bufs=2))
    accp = ctx.enter_context(tc.tile_pool(name="acc", bufs=2, space="PSUM"))
    tpp = ctx.enter_context(tc.tile_pool(name="tp", bufs=2, space="PSUM"))
    vpp = ctx.enter_context(tc.tile_pool(name="vp", bufs=2, space="PSUM"))

    idxs = cpool.tile([128, NB * 8], I16)
    nc.sync.dma_start(out=idxs[:], in_=ins["idx16w"][:])
    dl32 = cpool.tile([128, NB], F32)
    nc.sync.dma_start(out=dl32[:], in_=ins["dstloc"][:])
    dl = cpool.tile([128, NB], BF16)
    nc.vector.tensor_copy(out=dl[:], in_=dl32[:])
    deg = cpool.tile([128, NW], F32)
    nc.sync.dma_start(out=deg[:], in_=ins["deg_in_sh"][:])
    vm = cpool.tile([64, 128], F32)
    nc.sync.dma_start(out=vm[:RK, :DO], in_=ins["vmat"][:])
    ident = cpool.tile([128, 128], F32)
    make_identity(nc, ident[:])
    iota_i = cpool.tile([128, 128], I32)
    nc.gpsimd.iota(iota_i[:], pattern=[[1, 128]], base=0, channel_multiplier=0)
    iota_f = cpool.tile([128, 128], BF16)
    nc.vector.tensor_copy(out=iota_f[:], in_=iota_i[:])
    mask = cpool.tile([128, NW], F32)
    nc.vector.tensor_scalar(out=mask[:], in0=deg[:], scalar1=0.0, scalar2=None, op0=ALU.is_gt)
    inv = cpool.tile([128, NW], F32)
    nc.vector.tensor_scalar(out=inv[:], in0=deg[:], scalar1=1.0, scalar2=None, op0=ALU.max)
    nc.scalar.activation(out=inv[:], in_=inv[:], func=AF.Sqrt)
    nc.vector.reciprocal(out=inv[:], in_=inv[:])

    TLO = ins["T"][0:HI_BASE, 0:TCOL]
    THI = ins["T"][HI_BASE:T_ROWS, 0:TCOL] if any(BH[w] > 0 for w in range(NW)) else None

    # manually rotated gather buffers, zeroed once: stale/unwritten slots must
    # stay finite (0 * NaN would poison the routing matmul's PSUM column).
    Gbuf = cpool.tile([128, GBD, NBMAX, TCOL], BF16)
    nc.vector.memset(Gbuf[:], 0.0)

    out_r = outs["out_sh"].rearrange("(w p) f -> p w f", p=128)

    qload = [0] * NQUEUE

    w = 0
    while w < NW:
        ws = list(range(w, min(w + GROUP, NW)))
        nwg = len(ws)
        g0 = ws[0]
        H = hpool.tile([128, nwg, TCOL], F32, tag="H")
        for j, wi in enumerate(ws):
            nb = int(BL[wi] + BH[wi])
            b0 = int(woff[wi])
            G = Gbuf[:, wi % GBD]

            def _gather(tbl, blk_lo, blk_hi, ni_total, sec):
                # truncate to the real index count — stale tail slots map to
                # no dst slot (dstloc=-1 -> zero S row) and are finite.
                step = GATHER_CHUNK if GATHER_CHUNK > 0 else (blk_hi - blk_lo)
                done = 0
                for s in range(blk_lo, blk_hi, step):
                    e = min(s + step, blk_hi)
                    cap = (e - s) * 128
                    ni = min(max(ni_total - done, 0), cap)
                    done += cap
                    if ni == 0:
                        break
                    ni = max(ni, 16)
                    eb = s + (ni + 127) // 128
                    ni_reg = ni
                    q = min(range(NQUEUE), key=lambda i: qload[i])
                    qload[q] += ni
                    dma_gather_strided(
                        nc.gpsimd, out_ap=G[:, s:eb, :], in_ap=tbl,
                        idxs_ap=idxs[:, (b0 + s) * 8:(b0 + eb) * 8],
                        num_idxs=ni, num_idxs_reg=ni_reg,
                        elem_size=TCOL, elem_step=TROW,
                        single_packet=SINGLE_PACKET,
                        queue_num=q)

            _gather(TLO, 0, int(BL[wi]), int(cfg["ni_lo"][wi]), 0)
            if BH[wi] > 0:
                _gather(THI, int(BL[wi]), nb, int(cfg["ni_hi"][wi]), 1)
            S = spool.tile([128, NBMAX, 128], BF16, tag="S")
            nc.vector.tensor_tensor(
                out=S[:, :nb, :],
                in0=dl[:, b0:b0 + nb].unsqueeze(2).to_broadcast([128, nb, 128]),
                in1=iota_f[:].unsqueeze(1).to_broadcast([128, nb, 128]),
                op=ALU.is_equal)
            ps1 = accp.tile([128, TCOL], F32, tag="acc1")
            for b in range(nb):
                nc.tensor.matmul(out=ps1[:], lhsT=S[:, b, :], rhs=G[:, b, :],
                                 start=(b == 0), stop=(b == nb - 1))
            # scalar engine drains PSUM: DVE is the busier engine in phase B
            nc.scalar.activation(out=H[:, j, :], in_=ps1[:], func=AF.Copy)
        # epilogue for the group: unpack parities from the packed neg sums
        # (sum = cnt_a + 128*cnt_b, cnt_a < 128): ranks 0:32 parity = sum & 1,
        # ranks 32:64 parity = (sum >> 7) & 1.
        pi = epool.tile([128, GROUP, RK], I32, tag="pi")
        nc.vector.tensor_copy(out=pi[:, :nwg, 0:32], in_=H[:, :, DO + RK:TCOL])
        nc.vector.tensor_scalar(out=pi[:, :nwg, 32:64], in0=pi[:, :nwg, 0:32],
                                scalar1=7, scalar2=1,
                                op0=ALU.logical_shift_right, op1=ALU.bitwise_and)
        nc.vector.tensor_scalar(out=pi[:, :nwg, 0:32], in0=pi[:, :nwg, 0:32],
                                scalar1=1, scalar2=None, op0=ALU.bitwise_and)
        sg = epool.tile([128, GROUP, RK], F32, tag="sg")
        nc.vector.tensor_copy(out=sg[:, :nwg, :], in_=pi[:, :nwg, :])
        nc.vector.tensor_scalar(out=sg[:, :nwg, :], in0=sg[:, :nwg, :], scalar1=-2.0,
                                scalar2=1.0, op0=ALU.mult, op1=ALU.add)
        ex = epool.tile([128, GROUP, RK], F32, tag="ex")
        nc.scalar.activation(out=ex[:, :nwg, :], in_=H[:, :, DO:DO + RK], func=AF.Exp, scale=1.0)
        nc.vector.tensor_tensor(out=ex[:, :nwg, :], in0=ex[:, :nwg, :], in1=sg[:, :nwg, :], op=ALU.mult)
        nc.vector.tensor_tensor(
            out=ex[:, :nwg, :], in0=ex[:, :nwg, :],
            in1=mask[:, g0:g0 + nwg].unsqueeze(2).to_broadcast([128, nwg, RK]),
            op=ALU.mult)
        RST = rpool.tile([128, GROUP, DO], F32, tag="RST")
        for j, wi in enumerate(ws):
            tp = tpp.tile([64, 128], F32, tag="tp")
            nc.tensor.transpose(out=tp[:RK, :], in_=ex[:, j, :], identity=ident[:])
            hpT = epool.tile([64, 128], F32, tag="hpT")
            nc.vector.tensor_copy(out=hpT[:RK, :], in_=tp[:RK, :])
            vp = vpp.tile([128, DO], F32, tag="vpp")
            nc.tensor.matmul(out=vp[:], lhsT=hpT[:RK, :], rhs=vm[:RK, :DO], start=True, stop=True)
            nc.vector.tensor_tensor(out=RST[:, j, :], in0=H[:, j, 0:DO], in1=vp[:], op=ALU.add)
        nc.vector.tensor_tensor(
            out=RST[:, :nwg, :], in0=RST[:, :nwg, :],
            in1=inv[:, g0:g0 + nwg].unsqueeze(2).to_broadcast([128, nwg, DO]),
            op=ALU.mult)
        nc.sync.dma_start(out=out_r[:, g0:g0 + nwg, :], in_=RST[:, :nwg, :])
        w += GROUP


# ---------------- SPMD drivers ----------------

def _new_nc(n_cores):
    return bacc.Bacc("TRN2", target_bir_lowering=False, debug=False,
                     enable_asserts=False, num_devices=n_cores,
                     num_swdge_queues=NSWQ)


def _build_a(dm):
    nc = _new_nc(dm["M"])
    DIN, NSH, NW = dm["DIN"], dm["NSH"], dm["NW"]
    DT = dm["DOUT"] + dm["RANK"]
    ins = dict(
        featT=nc.dram_tensor("featT", [DIN, NSH], F32, kind="ExternalInput").ap(),
        Wcat=nc.dram_tensor("Wcat", [DIN, DT], F32, kind="ExternalInput").ap(),
        b2col=nc.dram_tensor("b2col", [dm["RANK"], 1], F32, kind="ExternalInput").ap(),
        deg_row=nc.dram_tensor("deg_row", [1, NSH], F32, kind="ExternalInput").ap(),
    )
    outs = dict(Tsh=nc.dram_tensor("Tsh", [TCOL, NSH], BF16, kind="ExternalOutput").ap())
    cfg = dict(NSH=NSH, NW=NW, DIN=DIN, DOUT=dm["DOUT"], RANK=dm["RANK"])
    with tile.TileContext(nc) as tc:
        build_phase_a(tc, outs, ins, cfg)
    nc.compile()
    return nc


def _build_b(dm, pp, group=None):
    group = GROUP_W if group is None else group
    nc = _new_nc(dm["M"])
    NW, NB = dm["NW"], pp["NB"]
    ins = dict(
        T=nc.dram_tensor("T", [dm["T_ROWS"], TROW], BF16, kind="ExternalInput").ap(),
        idx16w=nc.dram_tensor("idx16w", [128, NB * 8], I16, kind="ExternalInput").ap(),
        dstloc=nc.dram_tensor("dstloc", [128, NB], F32, kind="ExternalInput").ap(),
        deg_in_sh=nc.dram_tensor("deg_in_sh", [128, NW], F32, kind="ExternalInput").ap(),
        vmat=nc.dram_tensor("vmat", [dm["RANK"], dm["DOUT"]], F32, kind="ExternalInput").ap(),
        cnt=nc.dram_tensor("cnt", [1, NW * 2], I32, kind="ExternalInput").ap(),
    )
    outs = dict(out_sh=nc.dram_tensor("out_sh", [NW * 128, dm["DOUT"]], F32,
                                      kind="ExternalOutput").ap())
    cfg = dict(NW=NW, DOUT=dm["DOUT"], RANK=dm["RANK"], BL=pp["BL"], BH=pp["BH"],
               woff=pp["woff"], NB=NB, HI_BASE=dm["HI_BASE"], T_ROWS=dm["T_ROWS"],
               GROUP=group, ni_lo=pp["ni_lo"], ni_hi=pp["ni_hi"])
    with tile.TileContext(nc) as tc:
        build_phase_b(tc, outs, ins, cfg)
    nc.compile()
    return nc


def run_all(feat, w1, w2, v, src, dst, trace=False, tmpdir_a=None, tmpdir_b=None):
    """Returns (output, info dict with per-launch BassKernelResults)."""
    dm = make_dims(N=feat.shape[0], E=src.shape[0], DIN=feat.shape[1],
                   DOUT=w1.shape[1], RANK=v.shape[0])
    pp = preprocess(src, dst, dm)
    M, NSH = dm["M"], dm["NSH"]

    ains = host_phase_a_inputs(feat, w1, w2, pp, dm, pp["deg_out"])
    nc_a = _build_a(dm)
    ra = bass_utils.run_bass_kernel_spmd(nc_a, ains, list(range(M)), trace=trace,
                                         tmpdir=tmpdir_a)
    shards = [ra.results[k]["Tsh"] for k in range(M)]
    T = assemble_T(shards, dm)

    bins = host_phase_b_inputs(T, pp, v, dm)
    nc_b = _build_b(dm, pp)
    rb = bass_utils.run_bass_kernel_spmd(nc_b, bins, list(range(M)), trace=trace,
                                         tmpdir=tmpdir_b)
    out = np.concatenate([rb.results[k]["out_sh"][:NSH] for k in range(M)], axis=0)
    return out.astype(np.float32), dict(ra=ra, rb=rb, dm=dm, pp=pp)


def kernel(feat, w1, w2, v, src, dst):
    feat = np.asarray(feat, np.float32)
    w1 = np.asarray(w1, np.float32)
    w2 = np.asarray(w2, np.float32)
    v = np.asarray(v, np.float32)
    src = np.asarray(src)
    dst = np.asarray(dst)
    out, _ = run_all(feat, w1, w2, v, src, dst, trace=False)
    return out



# revision 51
# speedup vs baseline: 1.1489x; 1.1489x over previous
"""Trainium2 Bass kernel for nn_DGLGraphConv (graph conv with sum- and product-reduce).

Strategy (8 NeuronCores, SPMD, two launches):
  Launch A (node-sharded, 6250 nodes/core): per-node table, bf16, 256 cols:
      T[n] = [ (feat@w1)*s_out (128) | |log|tanh|| (64) | neg indicator (64) ]
  Host: concat T shards (pure relayout; row 0 / tail rows are zeros used as gather padding).
  Launch B (dst-sharded by edge partitioning): per core, 49 windows x 128 dst slots.
      Edges sorted by (dst window, src half, src), padded to 128-edge blocks with
      block counts maxed over cores so all cores run one program. Per block:
      dma_gather 128 rows of T (512B each), one-hot S matmul accumulates
      [sum(fs) | sum|ln| | count(neg)] per dst slot in PSUM; epilogue computes
      sign via parity, exp, mask, @v, and the in-degree scaling.

Host does integer index prep (bincount/sort/pad) and layout-only transforms;
all floating-point math runs on device.
"""
import sys
from contextlib import ExitStack

import numpy as np

for _p in ("/opt/trn_rl_repo",):
    if _p not in sys.path:
        sys.path.insert(0, _p)

import concourse.bass as bass
import concourse.mybir as mybir
import concourse.tile as tile
from concourse import bacc, bass_utils
from concourse._compat import with_exitstack
from concourse.masks import make_identity

F32 = mybir.dt.float32
I32 = mybir.dt.int32
I16 = mybir.dt.int16
BF16 = mybir.dt.bfloat16
AF = mybir.ActivationFunctionType
ALU = mybir.AluOpType

MASK_ABS = 0x7FFFFFFF
MASK_SGN = -0x80000000

# T table row stride (bf16 elems, 512B — gather table stride must be a
# multiple of 256B); payload is TCOL cols:
#   [fs(128) | absl(64) | negpk(32)]  negpk[k] = neg[k] + 128*neg[k+32]
TROW = 256
TCOL = 224
import os as _os
# Max 128-edge blocks per dma_gather call. Full-window calls (~2200 emitted
# descriptors) die on HW (descriptor-ring overflow); pad slots are emitted as
# trailing -1 (skipped by the ucode), keeping per-call emission ~<=1450.
GATHER_CHUNK = int(_os.environ.get("GCH", "6"))
SINGLE_PACKET = _os.environ.get("SPKT", "0") == "1"
# The gather ucode runs on Q7 core pair `queue_num` (cpu_id/2 == queue_num):
# 4 SWDGE queues = 4 concurrent descriptor-generation pairs. Balance calls
# over queues greedily by descriptor count.
NQUEUE = int(_os.environ.get("NQ", "4"))  # rotate gather calls over SWDGE queues
NSWQ = int(_os.environ.get("NSWQ", "4"))  # SWDGE queues allocated (1..4)
GROUP_W = int(_os.environ.get("GROUPW", "12"))  # windows per epilogue group
GBUFD = int(_os.environ.get("GBUFD", "4"))  # gather-buffer rotation depth
# windows whose G buffer may be uninitialized SBUF (NaN patterns): gather real
# zero rows for padding there instead of skipping, so 0*NaN never reaches PSUM.
PAD_REAL_WINDOWS = 3


# ---------------- host-side prep (integer/layout only) ----------------

def make_dims(N=50000, E=800000, DIN=256, DOUT=128, RANK=64, M=8, LO_MAX=32766):
    LO_MAX = min(LO_MAX, N - 1)
    NSH = N // M
    W = 128
    NW = (NSH + W - 1) // W
    T_ROWS = N + 4
    HI_BASE = LO_MAX + 2
    NHI = T_ROWS - HI_BASE
    return dict(N=N, E=E, DIN=DIN, DOUT=DOUT, RANK=RANK, M=M, NSH=NSH, W=W, NW=NW,
                LO_MAX=LO_MAX, T_ROWS=T_ROWS, HI_BASE=HI_BASE, NHI=NHI,
                HI_PAD_IDX=NHI - 2)


def preprocess(src, dst, dm):
    N, E, M, NSH, W, NW = dm["N"], dm["E"], dm["M"], dm["NSH"], dm["W"], dm["NW"]
    LO_MAX, HI_PAD_IDX = dm["LO_MAX"], dm["HI_PAD_IDX"]
    src = np.asarray(src).astype(np.int64)
    dst = np.asarray(dst).astype(np.int64)
    deg_out = np.bincount(src, minlength=N).astype(np.float32)
    deg_in = np.bincount(dst, minlength=N).astype(np.float32)

    order = np.lexsort((src, dst))
    s_srt, d_srt = src[order], dst[order]
    core_of = d_srt // NSH
    win_of = (d_srt - core_of * NSH) // W

    is_hi = s_srt > LO_MAX
    nlo = np.zeros((M, NW), np.int64)
    nhi = np.zeros((M, NW), np.int64)
    np.add.at(nlo, (core_of[~is_hi], win_of[~is_hi]), 1)
    np.add.at(nhi, (core_of[is_hi], win_of[is_hi]), 1)

    BL = np.maximum(1, -(-nlo.max(axis=0) // 128))
    BH = -(-nhi.max(axis=0) // 128)
    BT = BL + BH
    NB = int(BT.sum())

    idx16 = np.zeros((M, NB * 128), np.int16)
    dstloc = np.zeros((M, 128, NB), np.float32)
    key_half = is_hi.astype(np.int64)
    order2 = np.lexsort((s_srt, key_half, win_of, core_of))
    s2, d2 = s_srt[order2], d_srt[order2]
    c2, w2_, h2 = core_of[order2], win_of[order2], key_half[order2]

    woff = np.concatenate([[0], np.cumsum(BT)])
    counts = np.zeros((M, NW, 2), np.int64)
    np.add.at(counts, (c2, w2_, h2), 1)
    dstloc[:] = -1.0  # pad slots map to no dst slot (S one-hot row is zero)
    ptr = 0
    gch = GATHER_CHUNK if GATHER_CHUNK > 0 else 10**9
    for k in range(M):
        for w in range(NW):
            cl, ch = counts[k, w, 0], counts[k, w, 1]
            base_blk = woff[w]
            seg = slice(ptr, ptr + cl)
            pos = np.arange(cl)
            idx16[k, base_blk * 128 + pos] = (s2[seg] + 1).astype(np.int16)
            dstloc[k, pos % 128, base_blk + pos // 128] = (d2[seg] - k * NSH - w * W).astype(np.float32)
            ptr += cl
            pad_real = True  # -1 skip-pads crash the gather ucode on HW
            if cl < BL[w] * 128:
                pad = np.arange(cl, BL[w] * 128)
                idx16[k, base_blk * 128 + pad] = 0 if pad_real else -1
            base_blk_h = woff[w] + BL[w]
            seg = slice(ptr, ptr + ch)
            pos = np.arange(ch)
            idx16[k, base_blk_h * 128 + pos] = (s2[seg] - LO_MAX - 1).astype(np.int16)
            dstloc[k, pos % 128, base_blk_h + pos // 128] = (d2[seg] - k * NSH - w * W).astype(np.float32)
            ptr += ch
            if ch < BH[w] * 128:
                pad = np.arange(ch, BH[w] * 128)
                idx16[k, base_blk_h * 128 + pad] = HI_PAD_IDX if pad_real else -1
            # each of the 16 wrapped index lanes maps to one SDMA engine and
            # the DMA completion semaphore needs every engine to emit: keep
            # >=16 real (non-negative) indices at the head of every call.
            for sec_lo, sec_nb, fill in ((base_blk, int(BL[w]), 0),
                                         (base_blk_h, int(BH[w]), HI_PAD_IDX)):
                for s in range(0, sec_nb, gch):
                    e = min(s + gch, sec_nb)
                    sl = idx16[k, (sec_lo + s) * 128:(sec_lo + e) * 128]
                    if (sl[:16] < 0).any():
                        sl[:16][sl[:16] < 0] = fill
    assert ptr == E

    idx16_wrapped = np.tile(idx16.reshape(M, -1, 16).transpose(0, 2, 1), (1, 8, 1)).copy()

    def shard_deg(deg):
        out = np.zeros((M, 128, NW), np.float32)
        for k in range(M):
            d = deg[k * NSH:(k + 1) * NSH]
            d = np.concatenate([d, np.zeros(NW * W - NSH, np.float32)])
            out[k] = d.reshape(NW, W).T
        return out

    return dict(BL=BL, BH=BH, BT=BT, NB=NB, idx16_wrapped=idx16_wrapped, dstloc=dstloc,
                deg_in_sh=shard_deg(deg_in), deg_out=deg_out, woff=woff,
                ni_lo=counts[:, :, 0].max(axis=0), ni_hi=counts[:, :, 1].max(axis=0),
                cnt_own=counts.astype(np.int32))


def host_phase_a_inputs(feat, w1, w2, pp, dm, deg_out):
    M, NSH, DIN = dm["M"], dm["NSH"], dm["DIN"]
    featT = np.ascontiguousarray(np.asarray(feat).T)
    Wcat = np.ascontiguousarray(np.concatenate([w1, w2[:DIN]], axis=1), dtype=np.float32)
    b2col = np.ascontiguousarray(w2[DIN][:, None], dtype=np.float32)
    return [dict(featT=np.ascontiguousarray(featT[:, k * NSH:(k + 1) * NSH]),
                 Wcat=Wcat, b2col=b2col,
                 deg_row=np.ascontiguousarray(deg_out[None, k * NSH:(k + 1) * NSH]))
            for k in range(M)]


def assemble_T(shards, dm):
    # shards are [TCOL, NSH] (feature-major); T rows are nodes, padded to a
    # TROW (512B) stride — pure relayout.
    width = shards[0].shape[0]
    T = np.zeros((dm["T_ROWS"], TROW), shards[0].dtype)
    for k in range(dm["M"]):
        T[1 + k * dm["NSH"]:1 + (k + 1) * dm["NSH"], :width] = shards[k].T
    return T


def host_phase_b_inputs(T, pp, vmat, dm):
    NW = dm["NW"]
    # per-core real index counts per (window, section), clipped to the static
    # chunk geometry: >=16 keeps every SDMA lane's completion-sem leg alive.
    stat = np.stack([np.maximum(pp["ni_lo"], 16), np.maximum(pp["ni_hi"], 16)], axis=1)
    outs = []
    for k in range(dm["M"]):
        cnt = np.clip(pp["cnt_own"][k], 16, stat).astype(np.int32).reshape(1, NW * 2)
        outs.append(dict(T=T, idx16w=pp["idx16_wrapped"][k], dstloc=pp["dstloc"][k],
                         deg_in_sh=pp["deg_in_sh"][k],
                         vmat=np.ascontiguousarray(vmat, np.float32), cnt=cnt))
    return outs


# ---------------- device kernels ----------------

def dma_gather_strided(eng, out_ap, in_ap, idxs_ap, num_idxs, num_idxs_reg,
                       elem_size, elem_step, queue_num=0, single_packet=False):
    """nc.gpsimd.dma_gather minus its `elem_size_bytes % 256 == 0` assert.

    Only the table STRIDE (elem_step) must be a 256B multiple (the ucode
    encodes it as stride_bytes_256); the per-row payload moved can be any
    size <= stride. Mirrors the non-transpose DRAM-source tail of
    bass.dma_gather.
    """
    from concourse import ap_utils
    from concourse._compat import exact_div, round_up_to_multiple
    eng._assert_queue_num(queue_num)
    assert idxs_ap.dtype == mybir.dt.int16
    assert in_ap.dtype == out_ap.dtype
    assert ap_utils.ap_is_contiguous(out_ap.ap[1:])
    assert ap_utils.ap_is_contiguous(idxs_ap.ap[1:])
    assert in_ap.ap[-1][1] == out_ap.ap[-1][1] == elem_size
    assert out_ap.ap[0][1] * out_ap.ap[1][1] == round_up_to_multiple(num_idxs, 128)
    assert in_ap.ap[0][0] == elem_step
    stride_bytes = elem_step * mybir.dt.size(in_ap.dtype)
    stride_bytes_256 = exact_div(stride_bytes, 256)
    assert stride_bytes_256 < 256
    _in_ap = eng.lower_ap_dma(in_ap, for_custom_bir_dma=True)
    _idxs_ap = eng.lower_ap(idxs_ap)
    _out_ap = eng.lower_ap(out_ap)
    return eng.add_instruction(
        mybir.InstDMAGatherAnt(
            name=eng.bass.get_next_instruction_name(),
            ins=[*_in_ap, _idxs_ap, eng.lower_val_access(eng.to_reg(num_idxs_reg))],
            outs=[_out_ap],
            transpose=False,
            num_idxs=num_idxs,
            elem_size=elem_size,
            stride_bytes_256=stride_bytes_256,
            gen_mode=0,
            single_packet=single_packet,
            queue_num=queue_num,
            sbuf_tokens_per_rank=0,
            sbuf_free_dim_per_rank=0,
            sbuf_free_dim_pad_per_rank=0,
            sbuf_byte_offset=0,
        ))

@with_exitstack
def build_phase_a(ctx, tc, outs, ins, cfg):
    """Transposed orientation: features on partitions, nodes on the free dim.

    Output Tt [256, NSH] bf16: rows 0:128 fs = (feat*s)@w1, rows 128:192
    ln = log|tanh((feat*s)@w2 + b)| (signed, clamped >= -60), rows 192:256
    the tanh-negative indicator. Host transposes into T rows.
    """
    nc = tc.nc
    NSH = cfg["NSH"]
    DIN = cfg["DIN"]
    DO, RK = cfg["DOUT"], cfg["RANK"]
    DT = DO + RK
    KC = DIN // 128
    CW = 512
    NCHUNK = (NSH + CW - 1) // CW
    NLOAD = 4

    cpool = ctx.enter_context(tc.tile_pool(name="const", bufs=1))
    pp1 = ctx.enter_context(tc.tile_pool(name="ps1", bufs=3, space="PSUM"))
    pp2 = ctx.enter_context(tc.tile_pool(name="ps2", bufs=3, space="PSUM"))

    Wsb32 = cpool.tile([128, KC, DT], F32)
    nc.sync.dma_start(out=Wsb32[:],
                      in_=ins["Wcat"].rearrange("(c p) n -> p c n", p=128))
    Wsb = cpool.tile([128, KC, DT], BF16)
    nc.vector.tensor_copy(out=Wsb[:], in_=Wsb32[:])
    b2 = cpool.tile([64, 1], F32)
    nc.sync.dma_start(out=b2[:], in_=ins["b2col"][:])

    # out-degree scale: replicate deg to all partitions via a 0-stride DRAM
    # read, then clamp + Ln + Exp(-0.5x). Run in halves so the first half's
    # chain (which gates the first chunk multiplies) overlaps the second
    # half's broadcast DMA.
    srep = cpool.tile([128, NSH], F32)
    shalf = NSH // 2
    for h0, h1 in ((0, shalf), (shalf, NSH)):
        hs = slice(h0, h1)
        nc.sync.dma_start(out=srep[:, hs],
                          in_=ins["deg_row"][0:1, hs].to_broadcast([128, h1 - h0]))
        nc.vector.tensor_scalar(out=srep[:, hs], in0=srep[:, hs], scalar1=1.0,
                                scalar2=None, op0=ALU.max)
        nc.scalar.activation(out=srep[:, hs], in_=srep[:, hs], func=AF.Ln)
        nc.scalar.activation(out=srep[:, hs], in_=srep[:, hs], func=AF.Exp,
                             scale=-0.5)

    featT = cpool.tile([128, KC, NSH], BF16)
    step = (NSH + NLOAD - 1) // NLOAD
    featT_r = ins["featT"].rearrange("(c p) n -> p c n", p=128)
    for i in range(0, NSH, step):
        j = min(i + step, NSH)
        # SWDGE: only gpsimd DMA can cast f32->bf16 in flight
        nc.gpsimd.dma_start(out=featT[:, :, i:j], in_=featT_r[:, :, i:j])

    sfs = cpool.tile([128, NSH], BF16)
    sln = cpool.tile([64, NSH], BF16)
    sng = cpool.tile([64, NSH], BF16)
    thall = cpool.tile([64, NSH], F32)

    for q in range(NCHUNK):
        i0 = q * CW
        cw = min(CW, NSH - i0)
        sl = slice(i0, i0 + cw)
        ps1 = pp1.tile([128, CW], F32, tag="ps1")
        for c in range(KC):
            nc.tensor.matmul(out=ps1[:, :cw], lhsT=Wsb[:, c, 0:DO],
                             rhs=featT[:, c, sl], start=(c == 0), stop=(c == KC - 1))
        ps2 = pp2.tile([64, CW], F32, tag="ps2")
        for c in range(KC):
            nc.tensor.matmul(out=ps2[:, :cw], lhsT=Wsb[:, c, DO:DT],
                             rhs=featT[:, c, sl], start=(c == 0), stop=(c == KC - 1))
        nc.vector.tensor_tensor(out=sfs[:, sl], in0=ps1[:, :cw], in1=srep[:, sl],
                                op=ALU.mult)
        nc.vector.tensor_tensor(out=thall[:, sl], in0=ps2[:, :cw], in1=srep[0:64, sl],
                                op=ALU.mult)
        # tanh in-loop (single ACT table resident) so it overlaps the matmuls
        nc.scalar.activation(out=thall[:, sl], in_=thall[:, sl], func=AF.Tanh,
                             bias=b2[:])

    # sfs is final once the loop drains — write it while the log chain runs
    half = NSH // 2
    nc.sync.dma_start(out=outs["Tsh"][0:DO, 0:half], in_=sfs[:, 0:half])
    nc.sync.dma_start(out=outs["Tsh"][0:DO, half:NSH], in_=sfs[:, half:NSH])

    # hoisted log chain (one Ln table load), in halves so the stages and the
    # output DMAs pipeline; npk = a + 128*b ({0,1,128,129} bf16-exact).
    npk = cpool.tile([32, NSH], BF16)
    for h0, h1 in ((0, half), (half, NSH)):
        hs = slice(h0, h1)
        nc.vector.tensor_scalar(out=sng[:, hs], in0=thall[:, hs],
                                scalar1=0.0, scalar2=None, op0=ALU.is_lt)
        nc.vector.tensor_scalar(out=thall[:, hs].bitcast(I32),
                                in0=thall[:, hs].bitcast(I32),
                                scalar1=MASK_ABS, scalar2=None, op0=ALU.bitwise_and)
        nc.scalar.activation(out=thall[:, hs], in_=thall[:, hs], func=AF.Ln)
        nc.vector.tensor_scalar(out=sln[:, hs], in0=thall[:, hs],
                                scalar1=-60.0, scalar2=None, op0=ALU.max)
        nc.vector.tensor_scalar(out=npk[:, hs], in0=sng[32:64, hs], scalar1=128.0,
                                scalar2=None, op0=ALU.mult)
        nc.vector.tensor_tensor(out=npk[:, hs], in0=npk[:, hs], in1=sng[0:32, hs],
                                op=ALU.add)
        nc.sync.dma_start(out=outs["Tsh"][DO:DT, hs], in_=sln[:, hs])
        nc.sync.dma_start(out=outs["Tsh"][DT:TCOL, hs], in_=npk[:, hs])


@with_exitstack
def build_phase_b(ctx, tc, outs, ins, cfg):
    nc = tc.nc
    NW = cfg["NW"]
    DO, RK = cfg["DOUT"], cfg["RANK"]
    DT = DO + RK
    BL, BH = cfg["BL"], cfg["BH"]
    woff = cfg["woff"]
    NB = cfg["NB"]
    HI_BASE = cfg["HI_BASE"]
    T_ROWS = cfg["T_ROWS"]
    GROUP = cfg.get("GROUP", 12)
    GBD = GBUFD
    NBMAX = int(max(BL[w] + BH[w] for w in range(NW)))

    cpool = ctx.enter_context(tc.tile_pool(name="const", bufs=1))
    spool = ctx.enter_context(tc.tile_pool(name="s", bufs=2))
    hpool = ctx.enter_context(tc.tile_pool(name="h", bufs=2))
    epool = ctx.enter_context(tc.tile_pool(name="e", bufs=2))
    rpool = ctx.enter_context(tc.tile_pool(name="r", bufs=2))
    accp = ctx.enter_context(tc.tile_pool(name="acc", bufs=2, space="PSUM"))
    tpp = ctx.enter_context(tc.tile_pool(name="tp", bufs=2, space="PSUM"))
    vpp = ctx.enter_context(tc.tile_pool(name="vp", bufs=2, space="PSUM"))

    idxs = cpool.tile([128, NB * 8], I16)
    nc.sync.dma_start(out=idxs[:], in_=ins["idx16w"][:])
    dl32 = cpool.tile([128, NB], F32)
    nc.sync.dma_start(out=dl32[:], in_=ins["dstloc"][:])
    dl = cpool.tile([128, NB], BF16)
    nc.vector.tensor_copy(out=dl[:], in_=dl32[:])
    deg = cpool.tile([128, NW], F32)
    nc.sync.dma_start(out=deg[:], in_=ins["deg_in_sh"][:])
    vm = cpool.tile([64, 128], F32)
    nc.sync.dma_start(out=vm[:RK, :DO], in_=ins["vmat"][:])
    ident = cpool.tile([128, 128], F32)
    make_identity(nc, ident[:])
    iota_i = cpool.tile([128, 128], I32)
    nc.gpsimd.iota(iota_i[:], pattern=[[1, 128]], base=0, channel_multiplier=0)
    iota_f = cpool.tile([128, 128], BF16)
    nc.vector.tensor_copy(out=iota_f[:], in_=iota_i[:])
    mask = cpool.tile([128, NW], F32)
    nc.vector.tensor_scalar(out=mask[:], in0=deg[:], scalar1=0.0, scalar2=None, op0=ALU.is_gt)
    inv = cpool.tile([128, NW], F32)
    nc.vector.tensor_scalar(out=inv[:], in0=deg[:], scalar1=1.0, scalar2=None, op0=ALU.max)
    nc.scalar.activation(out=inv[:], in_=inv[:], func=AF.Sqrt)
    nc.vector.reciprocal(out=inv[:], in_=inv[:])

    TLO = ins["T"][0:HI_BASE, 0:TCOL]
    THI = ins["T"][HI_BASE:T_ROWS, 0:TCOL] if any(BH[w] > 0 for w in range(NW)) else None

    # manually rotated gather buffers, zeroed once: stale/unwritten slots must
    # stay finite (0 * NaN would poison the routing matmul's PSUM column).
    Gbuf = cpool.tile([128, GBD, NBMAX, TCOL], BF16)
    nc.vector.memset(Gbuf[:], 0.0)

    out_r = outs["out_sh"].rearrange("(w p) f -> p w f", p=128)

    qload = [0] * NQUEUE

    w = 0
    while w < NW:
        ws = list(range(w, min(w + GROUP, NW)))
        nwg = len(ws)
        g0 = ws[0]
        H = hpool.tile([128, nwg, TCOL], F32, tag="H")
        for j, wi in enumerate(ws):
            nb = int(BL[wi] + BH[wi])
            b0 = int(woff[wi])
            G = Gbuf[:, wi % GBD]

            def _gather(tbl, blk_lo, blk_hi, ni_total, sec):
                # truncate to the real index count — stale tail slots map to
                # no dst slot (dstloc=-1 -> zero S row) and are finite.
                step = GATHER_CHUNK if GATHER_CHUNK > 0 else (blk_hi - blk_lo)
                done = 0
                for s in range(blk_lo, blk_hi, step):
                    e = min(s + step, blk_hi)
                    cap = (e - s) * 128
                    ni = min(max(ni_total - done, 0), cap)
                    done += cap
                    if ni == 0:
                        break
                    ni = max(ni, 16)
                    eb = s + (ni + 127) // 128
                    ni_reg = ni
                    q = min(range(NQUEUE), key=lambda i: qload[i])
                    qload[q] += ni
                    dma_gather_strided(
                        nc.gpsimd, out_ap=G[:, s:eb, :], in_ap=tbl,
                        idxs_ap=idxs[:, (b0 + s) * 8:(b0 + eb) * 8],
                        num_idxs=ni, num_idxs_reg=ni_reg,
                        elem_size=TCOL, elem_step=TROW,
                        single_packet=SINGLE_PACKET,
                        queue_num=q)

            _gather(TLO, 0, int(BL[wi]), int(cfg["ni_lo"][wi]), 0)
            if BH[wi] > 0:
                _gather(THI, int(BL[wi]), nb, int(cfg["ni_hi"][wi]), 1)
            S = spool.tile([128, NBMAX, 128], BF16, tag="S")
            nc.vector.tensor_tensor(
                out=S[:, :nb, :],
                in0=dl[:, b0:b0 + nb].unsqueeze(2).to_broadcast([128, nb, 128]),
                in1=iota_f[:].unsqueeze(1).to_broadcast([128, nb, 128]),
                op=ALU.is_equal)
            ps1 = accp.tile([128, TCOL], F32, tag="acc1")
            for b in range(nb):
                nc.tensor.matmul(out=ps1[:], lhsT=S[:, b, :], rhs=G[:, b, :],
                                 start=(b == 0), stop=(b == nb - 1))
            # scalar engine drains PSUM: DVE is the busier engine in phase B
            nc.scalar.activation(out=H[:, j, :], in_=ps1[:], func=AF.Copy)
        # epilogue for the group: unpack parities from the packed neg sums
        # (sum = cnt_a + 128*cnt_b, cnt_a < 128): ranks 0:32 parity = sum & 1,
        # ranks 32:64 parity = (sum >> 7) & 1.
        pi = epool.tile([128, GROUP, RK], I32, tag="pi")
        nc.vector.tensor_copy(out=pi[:, :nwg, 0:32], in_=H[:, :, DO + RK:TCOL])
        nc.vector.tensor_scalar(out=pi[:, :nwg, 32:64], in0=pi[:, :nwg, 0:32],
                                scalar1=7, scalar2=1,
                                op0=ALU.logical_shift_right, op1=ALU.bitwise_and)
        nc.vector.tensor_scalar(out=pi[:, :nwg, 0:32], in0=pi[:, :nwg, 0:32],
                                scalar1=1, scalar2=None, op0=ALU.bitwise_and)
        sg = epool.tile([128, GROUP, RK], F32, tag="sg")
        nc.vector.tensor_copy(out=sg[:, :nwg, :], in_=pi[:, :nwg, :])
        nc.vector.tensor_scalar(out=sg[:, :nwg, :], in0=sg[:, :nwg, :], scalar1=-2.0,
                                scalar2=1.0, op0=ALU.mult, op1=ALU.add)
        ex = epool.tile([128, GROUP, RK], F32, tag="ex")
        nc.scalar.activation(out=ex[:, :nwg, :], in_=H[:, :, DO:DO + RK], func=AF.Exp, scale=1.0)
        nc.vector.tensor_tensor(out=ex[:, :nwg, :], in0=ex[:, :nwg, :], in1=sg[:, :nwg, :], op=ALU.mult)
        nc.vector.tensor_tensor(
            out=ex[:, :nwg, :], in0=ex[:, :nwg, :],
            in1=mask[:, g0:g0 + nwg].unsqueeze(2).to_broadcast([128, nwg, RK]),
            op=ALU.mult)
        RST = rpool.tile([128, GROUP, DO], F32, tag="RST")
        for j, wi in enumerate(ws):
            tp = tpp.tile([64, 128], F32, tag="tp")
            nc.tensor.transpose(out=tp[:RK, :], in_=ex[:, j, :], identity=ident[:])
            hpT = epool.tile([64, 128], F32, tag="hpT")
            nc.vector.tensor_copy(out=hpT[:RK, :], in_=tp[:RK, :])
            vp = vpp.tile([128, DO], F32, tag="vpp")
            nc.tensor.matmul(out=vp[:], lhsT=hpT[:RK, :], rhs=vm[:RK, :DO], start=True, stop=True)
            nc.vector.tensor_tensor(out=RST[:, j, :], in0=H[:, j, 0:DO], in1=vp[:], op=ALU.add)
        nc.vector.tensor_tensor(
            out=RST[:, :nwg, :], in0=RST[:, :nwg, :],
            in1=inv[:, g0:g0 + nwg].unsqueeze(2).to_broadcast([128, nwg, DO]),
            op=ALU.mult)
        nc.sync.dma_start(out=out_r[:, g0:g0 + nwg, :], in_=RST[:, :nwg, :])
        w += GROUP


# ---------------- SPMD drivers ----------------

def _new_nc(n_cores):
    return bacc.Bacc("TRN2", target_bir_lowering=False, debug=False,
                     enable_asserts=False, num_devices=n_cores,
                     num_swdge_queues=NSWQ)


def _build_a(dm):
    nc = _new_nc(dm["M"])
    DIN, NSH, NW = dm["DIN"], dm["NSH"], dm["NW"]
    DT = dm["DOUT"] + dm["RANK"]
    ins = dict(
        featT=nc.dram_tensor("featT", [DIN, NSH], F32, kind="ExternalInput").ap(),
        Wcat=nc.dram_tensor("Wcat", [DIN, DT], F32, kind="ExternalInput").ap(),
        b2col=nc.dram_tensor("b2col", [dm["RANK"], 1], F32, kind="ExternalInput").ap(),
        deg_row=nc.dram_tensor("deg_row", [1, NSH], F32, kind="ExternalInput").ap(),
    )
    outs = dict(Tsh=nc.dram_tensor("Tsh", [TCOL, NSH], BF16, kind="ExternalOutput").ap())
    cfg = dict(NSH=NSH, NW=NW, DIN=DIN, DOUT=dm["DOUT"], RANK=dm["RANK"])
    with tile.TileContext(nc) as tc:
        build_phase_a(tc, outs, ins, cfg)
    nc.compile()
    return nc


def _build_b(dm, pp, group=None):
    group = GROUP_W if group is None else group
    nc = _new_nc(dm["M"])
    NW, NB = dm["NW"], pp["NB"]
    ins = dict(
        T=nc.dram_tensor("T", [dm["T_ROWS"], TROW], BF16, kind="ExternalInput").ap(),
        idx16w=nc.dram_tensor("idx16w", [128, NB * 8], I16, kind="ExternalInput").ap(),
        dstloc=nc.dram_tensor("dstloc", [128, NB], F32, kind="ExternalInput").ap(),
        deg_in_sh=nc.dram_tensor("deg_in_sh", [128, NW], F32, kind="ExternalInput").ap(),
        vmat=nc.dram_tensor("vmat", [dm["RANK"], dm["DOUT"]], F32, kind="ExternalInput").ap(),
        cnt=nc.dram_tensor("cnt", [1, NW * 2], I32, kind="ExternalInput").ap(),
    )
    outs = dict(out_sh=nc.dram_tensor("out_sh", [NW * 128, dm["DOUT"]], F32,
                                      kind="ExternalOutput").ap())
    cfg = dict(NW=NW, DOUT=dm["DOUT"], RANK=dm["RANK"], BL=pp["BL"], BH=pp["BH"],
               woff=pp["woff"], NB=NB, HI_BASE=dm["HI_BASE"], T_ROWS=dm["T_ROWS"],
               GROUP=group, ni_lo=pp["ni_lo"], ni_hi=pp["ni_hi"])
    with tile.TileContext(nc) as tc:
        build_phase_b(tc, outs, ins, cfg)
    nc.compile()
    return nc


def run_all(feat, w1, w2, v, src, dst, trace=False, tmpdir_a=None, tmpdir_b=None):
    """Returns (output, info dict with per-launch BassKernelResults)."""
    dm = make_dims(N=feat.shape[0], E=src.shape[0], DIN=feat.shape[1],
                   DOUT=w1.shape[1], RANK=v.shape[0])
    pp = preprocess(src, dst, dm)
    M, NSH = dm["M"], dm["NSH"]

    ains = host_phase_a_inputs(feat, w1, w2, pp, dm, pp["deg_out"])
    nc_a = _build_a(dm)
    ra = bass_utils.run_bass_kernel_spmd(nc_a, ains, list(range(M)), trace=trace,
                                         tmpdir=tmpdir_a)
    shards = [ra.results[k]["Tsh"] for k in range(M)]
    T = assemble_T(shards, dm)

    bins = host_phase_b_inputs(T, pp, v, dm)
    nc_b = _build_b(dm, pp)
    rb = bass_utils.run_bass_kernel_spmd(nc_b, bins, list(range(M)), trace=trace,
                                         tmpdir=tmpdir_b)
    out = np.concatenate([rb.results[k]["out_sh"][:NSH] for k in range(M)], axis=0)
    return out.astype(np.float32), dict(ra=ra, rb=rb, dm=dm, pp=pp)


def kernel(feat, w1, w2, v, src, dst):
    feat = np.asarray(feat, np.float32)
    w1 = np.asarray(w1, np.float32)
    w2 = np.asarray(w2, np.float32)
    v = np.asarray(v, np.float32)
    src = np.asarray(src)
    dst = np.asarray(dst)
    out, _ = run_all(feat, w1, w2, v, src, dst, trace=False)
    return out

